# revision 35
# baseline (speedup 1.0000x reference)
"""GCN layer on 8 trn2 NeuronCores -- flat-stream bf16 gather pipeline.

out[r] = (sum_{e:row[e]=r} dis[row_e]*dis[col_e] * x[col_e]) @ W + bias,
dis = rsqrt(1 + outdeg), self-loops included as ordinary edges.

Sharding: destination nodes partitioned across 8 cores (12500 each),
degree-serpentine balanced; each core independent (x replicated; no
collectives). Host unshards via the (core, rloc) permutation.

Device algorithm per core (norm fully factored out of the edge stream):
  - X' = x * dis[col] in bf16 (host). dis[row] applied at the output stage,
    so the edge stream carries no per-edge scaling at all.
  - Edges land in 4 flat gather streams by col-block of 32768 (int16 idx
    limit), sorted by dest; slots are a per-(512-dest superblock, stream)
    grid equalized across cores so one NEFF serves all 8. Padding ~1.7%.
  - dma_gather in 3584-edge batches (amortizes the 994ns SWDGE overhead;
    64KB/partition SWDGE scratch -- an 8KB ring silently corrupts on HW)
    pulls 256B bf16 rows into [128, 28, 128] slabs, 2 buffers per stream.
  - Per 128-slot chunk and touched dest block: S = (iota == rowloc) 0/1
    indicator bf16, built 8 chunks per DVE tensor_tensor (d-major layout
    keeps every operand's last dim stride-1 so DVE runs in 2x mode; the
    rowloc broadcast rides the middle dim). Chunks may straddle block
    boundaries (the straddling chunk matmuls twice) -- no per-group chunk
    rounding, so DMA pays only real slots.
  - PE accumulates aggT[f,d] += slab.T @ S in PSUM per 128-dest block.
  - Drain per block: aggT->bf16 (Act), po = aggT.T @ W (PE),
    ob = po*dis_d + bias (one DVE scalar_tensor_tensor), DMA to y.

Cost-model floor: the gather descriptors (256B -> 22.76ns each, /16 DMA
engines) put the DMA device at ~597us busy; DVE ~50%, PE ~47%, Pool ~39%.
The GPSIMD ap_gather offload path was explored and abandoned: its cost is
max(num_idxs, num_elems) per call, which the SBUF-resident table dominates.
"""

import numpy as np
import ml_dtypes

import concourse.bass as bass
import concourse.mybir as mybir
import concourse.tile as tile
from concourse import bacc
from concourse import library_config
from concourse.bass_utils import run_bass_kernel_spmd

F = 128
CH = 128          # edges per chunk (slab partition dim)
N_CORES = 8
NPC = 12500       # dest nodes per core
DBLK = 128        # dest block width (psum tile)
NBLK = (NPC + DBLK - 1) // DBLK   # 98
SUP = 512         # superblock of dests (grid granularity)
NSUP = (NPC + SUP - 1) // SUP     # 25
KBLK = 32768      # dma-path col blocking (int16 idx limit)
NKD = 4           # dma streams
RES_ELEMS = 24576  # ap_gather num_elems per half (f32 pair elements)
RES_NODES = 2 * RES_ELEMS  # 65536 cols covered by the resident
SB = 8            # chunks per S8 tensor_tensor batch
GB = 3584         # edges per dma_gather batch
PB = 1024         # edges per ap_gather batch (8 transpose windows)


def _prep(x, edge_index, weight, bias, pool_frac):
    """Host-side routing/index prep. Returns (shared structure, per-core arrays)."""
    n = x.shape[0]
    r = np.asarray(edge_index[0], dtype=np.int64)
    c = np.asarray(edge_index[1], dtype=np.int64)
    deg = (np.bincount(r, minlength=n) + 1).astype(np.float64)
    dis = (1.0 / np.sqrt(deg)).astype(np.float32)

    loops = np.arange(n, dtype=np.int64)
    rr = np.concatenate([r, loops])
    cc = np.concatenate([c, loops])

    xs = (x * dis[:, None]).astype(ml_dtypes.bfloat16)   # X' [n, 128] bf16
    # pair-packed resident: [128, RES_ELEMS] f32; partition p (=64*h+q) holds
    # f32 word q of X'[u] for nodes u in half h's range split across q... see
    # below: half h serves cols [h*32768, (h+1)*32768), channels h*64..h*64+63,
    # element u = node h*32768+u, partition q = feature pair q.
    resv = np.zeros((128, RES_ELEMS), dtype=np.float32)
    for h in range(2):
        blk = xs[h * RES_ELEMS:(h + 1) * RES_ELEMS].view(np.float32)
        resv[h * 64:(h + 1) * 64, :] = blk.T  # [64, RES_ELEMS]

    # degree-balanced dest -> (core, rloc) assignment: serpentine by degree
    # equalizes per-(core, sup, stream) counts so the cross-core max grid
    # carries ~1% padding instead of ~4.6%
    order_d = np.argsort(-deg, kind="stable")
    pos = np.arange(n)
    colc = pos % N_CORES
    cidx = np.where((pos // N_CORES) % 2 == 0, colc, N_CORES - 1 - colc)
    cmap = np.empty(n, np.int64)
    cmap[order_d] = cidx
    rmap = np.empty(n, np.int64)
    for c_ in range(N_CORES):
        mine = order_d[cidx == c_]
        bpos = np.arange(len(mine))
        bcol = bpos % NBLK
        brow = bpos // NBLK
        bidx = np.where(brow % 2 == 0, bcol, NBLK - 1 - bcol)
        rmap[mine] = bidx * DBLK + brow
    core = cmap[rr]
    rloc = rmap[rr]
    invmap = np.full((N_CORES, NBLK * DBLK), -1, np.int64)
    invmap[cmap, rmap] = np.arange(n)

    NS = NKD + 2
    # per (core, sup, stream) edge lists
    order = np.lexsort((cc, rloc, core))
    rr_s, cc_s, rl_s, co_s = rr[order], cc[order], rloc[order], core[order]
    sup_s = rl_s // SUP

    rng = np.random.RandomState(12345)
    kblk = cc_s // KBLK
    # sup 0 stays DMA-only so the pool path never stalls on the resident load
    elig = (cc_s < RES_NODES) & (sup_s >= 1)
    topool = elig & (rng.rand(len(cc_s)) < pool_frac)
    stream = np.where(topool, NKD + cc_s // RES_ELEMS,
                      np.minimum(kblk, NKD - 1))
    # (col >= 98304 goes to k=3 stream; idx offset handled per stream)

    # counts [core, sup, stream]
    idx3 = (co_s * NSUP + sup_s) * NS + stream
    cnt = np.bincount(idx3, minlength=N_CORES * NSUP * NS).reshape(
        N_CORES, NSUP, NS)
    grid = cnt.max(axis=0)          # [NSUP, NS] shared slot allocation
    base = np.zeros((NSUP, NS), np.int64)
    base[1:, :] = np.cumsum(grid, axis=0)[:-1, :]
    L = grid.sum(axis=0)            # stream lengths
    # pad stream lengths: dma to %128, pool to %PB
    Lp = np.empty(NS, np.int64)
    for s in range(NS):
        m = CH if s < NKD else PB
        Lp[s] = max(((L[s] + m - 1) // m) * m, m)

    nchunks = [int(Lp[s] // CH) for s in range(NS)]

    # per-core slot assignment (position within (core, sup, stream) group,
    # preserving the rloc-sorted order within each group)
    gstart = np.zeros(N_CORES * NSUP * NS, np.int64)
    gcnt = np.bincount(idx3, minlength=N_CORES * NSUP * NS)
    gstart[1:] = np.cumsum(gcnt)[:-1]
    order2 = np.argsort(idx3, kind="stable")
    within = np.empty(len(rr_s), np.int64)
    within[order2] = np.arange(len(rr_s)) - gstart[idx3[order2]]
    slot = base[sup_s, stream] + within   # slot within stream

    # events: (stream, chunk, block) union over cores
    # per-core b-ranges per (stream, chunk): compute via edge slots
    bset = set()
    b_s = rl_s // DBLK
    chunk_of = slot // CH
    for key in zip(stream, chunk_of, b_s):
        bset.add(key)
    events = sorted(bset, key=lambda t: (t[2], t[0], t[1]))  # by (b, s, ci)
    NEV = len(events)
    NEVp = ((NEV + SB - 1) // SB) * SB

    # first/last event index per block
    ev_of_b = {}
    for j, (s, ci, b) in enumerate(events):
        ev_of_b.setdefault(b, []).append(j)

    # per-core device arrays
    per_core = []
    ev_index = {(s, ci, b): j for j, (s, ci, b) in enumerate(events)}
    for ci_ in range(N_CORES):
        sel = co_s == ci_
        st_c, sl_c, cc_c, rl_c, b_c = (stream[sel], slot[sel], cc_s[sel],
                                       rl_s[sel], b_s[sel])
        idx_arrs = []
        for s in range(NS):
            arr = np.zeros(int(Lp[s]), np.int16)
            m = st_c == s
            off = (RES_ELEMS * (s - NKD) if s >= NKD
                   else KBLK * min(s, NKD - 1))
            v = cc_c[m] - off
            if s == NKD - 1:  # k=3 stream also holds col >= 98304
                v = np.minimum(v, KBLK - 1)  # safety; cols < 100000-98304+32768 ok
            arr[sl_c[m]] = v.astype(np.int16)
            wrapped = arr.reshape(-1, 16).T.copy()  # [16, Lp/16]
            if s < NKD:
                # ship [16, n] as f32; PE broadcasts to 128 partitions on-chip
                idx_arrs.append(np.ascontiguousarray(
                    wrapped.astype(np.float32)))
            else:
                idx_arrs.append(np.tile(wrapped, (4, 1)).copy())

        rowloc = np.full((NEVp, CH), -1.0, dtype=np.float32)
        jj = np.array([ev_index[(s_, sl_ // CH, b_)]
                       for s_, sl_, b_ in zip(st_c, sl_c, b_c)])
        rowloc[jj, sl_c % CH] = (rl_c - b_c * DBLK).astype(np.float32)
        inv_c = invmap[ci_]
        disfull = np.where(inv_c >= 0, dis[np.maximum(inv_c, 0)], 0.0)
        disdst = disfull.reshape(NBLK, DBLK).T.astype(np.float32).copy()

        per_core.append({
            "rowloc": np.ascontiguousarray(
                rowloc.T.astype(ml_dtypes.bfloat16)),  # [128, NEVp] bf16
            "disdst": disdst,                          # [128, NBLK] f32
            **{f"idx{s}": idx_arrs[s] for s in range(NS)},
        })

    iotab = np.repeat(np.arange(DBLK, dtype=np.float32), SB)[None, :]
    iotab = np.tile(iotab, (128, 1)).astype(ml_dtypes.bfloat16)  # d-major
    shared = {
        "xs": np.ascontiguousarray(xs),
        "resv": resv,
        "wb": weight.astype(ml_dtypes.bfloat16),
        "iotab": iotab,
        "biasrep": np.tile(bias.astype(np.float32)[None, :], (DBLK, 1)),
        "eye64": np.tile(np.eye(64, dtype=np.float32), (2, 1)),
        "sel16": np.ascontiguousarray(
            np.tile(np.eye(16, dtype=np.float32), (1, 8))),
    }
    struct = {
        "events": events, "NEV": NEV, "NEVp": NEVp, "Lp": Lp,
        "nchunks": nchunks, "NS": NS, "cmap": cmap, "rmap": rmap,
    }
    return struct, shared, per_core


def _build(struct):
    events, NEVp, Lp, nchunks, NS = (struct["events"], struct["NEVp"],
                                     struct["Lp"], struct["nchunks"],
                                     struct["NS"])
    nc = bacc.Bacc(None, target_bir_lowering=False,
                   dynamic_dma_scratch_size=65536)
    dt = mybir.dt

    xs_d = nc.dram_tensor("xs", [100000, F], dt.bfloat16, kind="ExternalInput")
    resv_d = nc.dram_tensor("resv", [128, RES_ELEMS], dt.float32,
                            kind="ExternalInput")
    w_d = nc.dram_tensor("wb", [F, F], dt.bfloat16, kind="ExternalInput")
    iota_d = nc.dram_tensor("iotab", [128, SB * DBLK], dt.bfloat16,
                            kind="ExternalInput")
    bias_d = nc.dram_tensor("biasrep", [DBLK, F], dt.float32,
                            kind="ExternalInput")
    rowloc_d = nc.dram_tensor("rowloc", [128, NEVp], dt.bfloat16,
                              kind="ExternalInput")
    disdst_d = nc.dram_tensor("disdst", [DBLK, NBLK], dt.float32,
                              kind="ExternalInput")
    eye_d = nc.dram_tensor("eye64", [128, 64], dt.float32,
                           kind="ExternalInput")
    sel_d = nc.dram_tensor("sel16", [16, 128], dt.float32,
                           kind="ExternalInput")
    idx_d = [nc.dram_tensor(f"idx{s}", [16, int(Lp[s]) // 16], dt.float32,
                            kind="ExternalInput") if s < NKD else
             nc.dram_tensor(f"idx{s}", [64, int(Lp[s]) // 16], dt.int16,
                            kind="ExternalInput")
             for s in range(NS)]
    y_d = nc.dram_tensor("y", [NBLK * DBLK, F], dt.float32,
                         kind="ExternalOutput")

    CPB_D = GB // CH   # dma chunks per slab batch
    CPB_P = PB // CH   # pool chunks per batch
    have_pool = any(s >= NKD for s, _, _ in events)

    with tile.TileContext(nc) as tc:
        with (
            tc.tile_pool(name="const", bufs=1) as constp,
            tc.tile_pool(name="slab", bufs=2) as slabp,
            tc.tile_pool(name="idxp", bufs=2) as idxp,
            tc.tile_pool(name="idxfp", bufs=2) as idxfp,
            tc.tile_pool(name="ibcp", bufs=2, space="PSUM") as ibcp,
            tc.tile_pool(name="pidxp", bufs=2) as pidxp,
            tc.tile_pool(name="slabt", bufs=2) as slabtp,
            tc.tile_pool(name="slab8", bufs=3) as slab8p,
            tc.tile_pool(name="sp", bufs=3) as sp_,
            tc.tile_pool(name="pre", bufs=2) as prep,
            tc.tile_pool(name="ob", bufs=2) as obp,
            tc.tile_pool(name="ptr", bufs=2, space="PSUM") as ptrp,
            tc.tile_pool(name="pagg", bufs=4, space="PSUM") as paggp,
            tc.tile_pool(name="pout", bufs=2, space="PSUM") as poutp,
        ):
            if have_pool:
                nc.gpsimd.load_library(library_config.ap_gather)
            x_src = [xs_d[min(s, NKD - 1) * KBLK:
                          min((min(s, NKD - 1) + 1) * KBLK, 100000), :]
                     for s in range(NKD)]

            sel_sb = constp.tile([16, 128], dt.float32, tag="sel16")
            nc.sync.dma_start(sel_sb[:], sel_d[:])

            # stream state
            nbatch_done = [0] * NS
            slab_tiles = {}   # (s, batch) -> (tile, kind)

            def ensure_batch(s, bi, prefetch=True):
                if nbatch_done[s] > bi:
                    return
                assert nbatch_done[s] == bi, (s, bi, nbatch_done[s])
                nbatch_done[s] += 1
                nbatches_s = (int(Lp[s]) + (GB if s < NKD else PB) - 1) \
                    // (GB if s < NKD else PB)
                if s < NKD:
                    n_idx = min(GB, int(Lp[s]) - bi * GB)
                    n_ch = (n_idx + CH - 1) // CH
                    slab = slabp.tile([128, CPB_D, F], dt.bfloat16,
                                      tag=f"slab{s}")
                    itf = idxfp.tile([16, GB // 16], dt.float32,
                                     tag=f"idxf{s}")
                    nc.sync.dma_start(
                        itf[:, :n_idx // 16],
                        idx_d[s][:, bi * (GB // 16):
                                 bi * (GB // 16) + n_idx // 16])
                    ibc = ibcp.tile([128, GB // 16], dt.float32, tag="ibc")
                    nc.tensor.matmul(ibc[:, :n_idx // 16], sel_sb[:],
                                     itf[:, :n_idx // 16],
                                     start=True, stop=True)
                    it = idxp.tile([128, GB // 16], dt.int16, tag=f"idx{s}")
                    nc.vector.tensor_copy(it[:, :n_idx // 16],
                                          ibc[:, :n_idx // 16])
                    nc.gpsimd.dma_gather(
                        slab[:, :n_ch, :], x_src[s], it[:, :n_idx // 16],
                        n_idx, n_idx, F, single_packet=False)
                    slab_tiles[(s, bi)] = slab
                    if prefetch and bi + 1 < nbatches_s:
                        ensure_batch(s, bi + 1, prefetch=False)
                else:
                    h = s - NKD
                    pt_full = pidxp.tile([128, PB // 16], dt.int16,
                                         tag=f"pidx{h}")
                    pt = pt_full[h * 64:(h + 1) * 64, :]
                    nc.sync.dma_start(
                        pt, idx_d[s][:, bi * (PB // 16):
                                     (bi + 1) * (PB // 16)])
                    st = slabtp.tile([128, PB], dt.float32, tag=f"slabt{h}")
                    sl = st[h * 64:(h + 1) * 64, :]
                    nc.gpsimd.ap_gather(
                        sl, res_sb[h * 64:(h + 1) * 64, :], pt,
                        64, RES_ELEMS, 1, PB)
                    ptr = ptrp.tile([128, CPB_P, 64], dt.float32,
                                    tag="ptr")
                    for w in range(CPB_P):
                        nc.tensor.transpose(
                            ptr[:, w, :], sl[:, w * CH:(w + 1) * CH],
                            eye_sb[h * 64:(h + 1) * 64, :])
                    s8 = slab8p.tile([128, CPB_P * 64], dt.float32,
                                     tag=f"s8_{h}")
                    nc.scalar.activation(
                        s8[:], ptr[:].rearrange("p a b -> p (a b)"),
                        mybir.ActivationFunctionType.Copy)
                    slab_tiles[(s, bi)] = s8

            def slab_slice(s, ci):
                cpb = CPB_D if s < NKD else CPB_P
                bi = ci // cpb
                t = slab_tiles[(s, bi)]
                if s < NKD:
                    return t[:, ci % cpb, :]
                w = ci % cpb
                return t[:, w * 64:(w + 1) * 64].bitcast(dt.bfloat16)

            # prefetch the first gather batch of every dma stream so the
            # DMA device starts on edge data, not consts
            for s_ in range(NKD):
                if int(Lp[s_]) > 0 and nchunks[s_] > 0:
                    ensure_batch(s_, 0)

            w_sb = constp.tile([F, F], dt.bfloat16, tag="w")
            nc.sync.dma_start(w_sb[:], w_d[:])
            iota_sb = constp.tile([128, SB * DBLK], dt.bfloat16, tag="iota")
            nc.sync.dma_start(iota_sb[:], iota_d[:])
            bias_sb = constp.tile([DBLK, F], dt.float32, tag="bias")
            nc.sync.dma_start(bias_sb[:], bias_d[:])
            rowloc_sb = constp.tile([128, NEVp], dt.bfloat16, tag="rowloc")
            nc.sync.dma_start(rowloc_sb[:], rowloc_d[:])
            disdst_sb = constp.tile([DBLK, NBLK], dt.float32, tag="disdst")
            nc.sync.dma_start(disdst_sb[:], disdst_d[:])
            if have_pool:
                res_sb = constp.tile([128, RES_ELEMS], dt.float32, tag="res")
                nc.sync.dma_start(res_sb[:], resv_d[:])
                eye_sb = constp.tile([128, 64], dt.float32, tag="eye")
                nc.sync.dma_start(eye_sb[:], eye_d[:])


            # block -> event index span (events sorted by (b, s, ci))
            ev_b = [e[2] for e in events]
            s8_tiles = {}

            def ensure_s8(g):
                if g in s8_tiles:
                    return
                t = sp_.tile([128, DBLK * SB], dt.bfloat16, tag="s8t")
                nc.vector.tensor_tensor(
                    t[:].rearrange("p (a b) -> p a b", a=DBLK),
                    iota_sb[:].rearrange("p (a b) -> p a b", a=DBLK),
                    rowloc_sb[:, g * SB:(g + 1) * SB]
                    .unsqueeze(1).broadcast_to([128, DBLK, SB]),
                    mybir.AluOpType.is_equal)
                s8_tiles[g] = t

            j = 0
            NEV = struct["NEV"]
            while j < NEV:
                b = ev_b[j]
                j_end = j
                while j_end < NEV and ev_b[j_end] == b:
                    j_end += 1
                # pre-pass: all gathers/transposes/copies/S-builds for this
                # block BEFORE opening the PE accumulation group (a PE
                # transpose inside an open group deadlocks the PE order)
                for jj in range(j, j_end):
                    s, ci, _ = events[jj]
                    cpb = CPB_D if s < NKD else CPB_P
                    ensure_batch(s, ci // cpb)
                pa = paggp.tile([128, DBLK], dt.float32, tag="pagg")
                for jj in range(j, j_end):
                    s, ci, _ = events[jj]
                    ensure_s8(jj // SB)
                    st8 = s8_tiles[jj // SB]
                    nc.tensor.matmul(
                        pa[:], slab_slice(s, ci),
                        st8[:].rearrange("p (a b) -> p a b", a=DBLK)
                        [:, :, jj % SB],
                        start=(jj == j), stop=(jj == j_end - 1))
                # drain block b
                pre = prep.tile([128, DBLK], dt.bfloat16, tag="pre")
                nc.scalar.activation(pre[:], pa[:],
                                     mybir.ActivationFunctionType.Copy)
                po = poutp.tile([DBLK, F], dt.float32, tag="po")
                nc.tensor.matmul(po[:], pre[:], w_sb[:], start=True,
                                 stop=True)
                ob = obp.tile([DBLK, F], dt.float32, tag="ob")
                nc.vector.scalar_tensor_tensor(
                    ob[:], po[:], disdst_sb[:, b:b + 1], bias_sb[:],
                    op0=mybir.AluOpType.mult, op1=mybir.AluOpType.add)
                nc.sync.dma_start(y_d[b * DBLK:(b + 1) * DBLK, :], ob[:])
                j = j_end

    nc.compile()
    return nc


def kernel(x, edge_index, weight, bias, _pool_frac=0.0, _return_nc=False):
    x = np.ascontiguousarray(np.asarray(x, dtype=np.float32))
    edge_index = np.asarray(edge_index)
    weight = np.ascontiguousarray(np.asarray(weight, dtype=np.float32))
    bias = np.asarray(bias, dtype=np.float32)
    n = x.shape[0]
    assert n == 100000 and n % N_CORES == 0

    struct, shared, per_core = _prep(x, edge_index, weight, bias, _pool_frac)
    nc = _build(struct)

    in_maps = [{**shared, **per_core[ci]} for ci in range(N_CORES)]
    res = run_bass_kernel_spmd(nc, in_maps, core_ids=list(range(N_CORES)))
    ys = np.stack([np.asarray(res.results[ci]["y"]).astype(np.float32)
                   for ci in range(N_CORES)])
    out = ys[struct["cmap"], struct["rmap"]]
    if _return_nc:
        return out, nc, in_maps
    return out


# revision 36
# speedup vs baseline: 1.0395x; 1.0395x over previous
"""GCN layer on 8 trn2 NeuronCores -- flat-stream bf16 gather pipeline.

out[r] = (sum_{e:row[e]=r} dis[row_e]*dis[col_e] * x[col_e]) @ W + bias,
dis = rsqrt(1 + outdeg), self-loops included as ordinary edges.

Sharding: destination nodes partitioned across 8 cores (12500 each),
degree-serpentine balanced; each core independent (x replicated; no
collectives). Host unshards via the (core, rloc) permutation.

Device algorithm per core (norm fully factored out of the edge stream):
  - X' = x * dis[col] in bf16 (host). dis[row] applied at the output stage,
    so the edge stream carries no per-edge scaling at all.
  - Edges land in 4 flat gather streams by col-block of 32768 (int16 idx
    limit), sorted by dest; slots are a per-(512-dest superblock, stream)
    grid equalized across cores so one NEFF serves all 8. Padding ~1.7%.
  - dma_gather in 3584-edge batches (amortizes the 994ns SWDGE overhead;
    64KB/partition SWDGE scratch -- an 8KB ring silently corrupts on HW)
    pulls 256B bf16 rows into [128, 28, 128] slabs, 2 buffers per stream.
  - Per 128-slot chunk and touched dest block: S = (iota == rowloc) 0/1
    indicator bf16, built 8 chunks per DVE tensor_tensor (d-major layout
    keeps every operand's last dim stride-1 so DVE runs in 2x mode; the
    rowloc broadcast rides the middle dim). Chunks may straddle block
    boundaries (the straddling chunk matmuls twice) -- no per-group chunk
    rounding, so DMA pays only real slots.
  - PE accumulates aggT[f,d] += slab.T @ S in PSUM per 128-dest block.
  - Drain per block: aggT->bf16 (Act), po = aggT.T @ W (PE),
    ob = po*dis_d + bias (one DVE scalar_tensor_tensor), DMA to y.

Cost-model floor: the gather descriptors (256B -> 22.76ns each, /16 DMA
engines) put the DMA device at ~597us busy; DVE ~50%, PE ~47%, Pool ~39%.
The GPSIMD ap_gather offload path was explored and abandoned: its cost is
max(num_idxs, num_elems) per call, which the SBUF-resident table dominates.
"""

import numpy as np
import ml_dtypes

import concourse.bass as bass
import concourse.mybir as mybir
import concourse.tile as tile
from concourse import bacc
from concourse import library_config
from concourse.bass_utils import run_bass_kernel_spmd

F = 128
CH = 128          # edges per chunk (slab partition dim)
N_CORES = 8
NPC = 12500       # dest nodes per core
DBLK = 128        # dest block width (psum tile)
NBLK = (NPC + DBLK - 1) // DBLK   # 98
SUP = 512         # superblock of dests (grid granularity)
NSUP = (NPC + SUP - 1) // SUP     # 25
KBLK = 32768      # dma-path col blocking (int16 idx limit)
NKD = 4           # dma streams
RES_ELEMS = 24576  # ap_gather num_elems per half (f32 pair elements)
RES_NODES = 2 * RES_ELEMS  # 65536 cols covered by the resident
SB = 8            # chunks per S8 tensor_tensor batch
GB = 3584         # edges per dma_gather batch
PB = 1024         # edges per ap_gather batch (8 transpose windows)


def _prep(x, edge_index, weight, bias, pool_frac):
    """Host-side routing/index prep. Returns (shared structure, per-core arrays)."""
    n = x.shape[0]
    r = np.asarray(edge_index[0], dtype=np.int64)
    c = np.asarray(edge_index[1], dtype=np.int64)
    deg = (np.bincount(r, minlength=n) + 1).astype(np.float64)
    dis = (1.0 / np.sqrt(deg)).astype(np.float32)

    loops = np.arange(n, dtype=np.int64)
    rr = np.concatenate([r, loops])
    cc = np.concatenate([c, loops])

    xs = (x * dis[:, None]).astype(ml_dtypes.bfloat16)   # X' [n, 128] bf16
    # pair-packed resident: [128, RES_ELEMS] f32; partition p (=64*h+q) holds
    # f32 word q of X'[u] for nodes u in half h's range split across q... see
    # below: half h serves cols [h*32768, (h+1)*32768), channels h*64..h*64+63,
    # element u = node h*32768+u, partition q = feature pair q.
    resv = np.zeros((128, RES_ELEMS), dtype=np.float32)
    for h in range(2):
        blk = xs[h * RES_ELEMS:(h + 1) * RES_ELEMS].view(np.float32)
        resv[h * 64:(h + 1) * 64, :] = blk.T  # [64, RES_ELEMS]

    # degree-balanced dest -> (core, rloc) assignment: serpentine by degree
    # equalizes per-(core, sup, stream) counts so the cross-core max grid
    # carries ~1% padding instead of ~4.6%
    order_d = np.argsort(-deg, kind="stable")
    pos = np.arange(n)
    colc = pos % N_CORES
    cidx = np.where((pos // N_CORES) % 2 == 0, colc, N_CORES - 1 - colc)
    cmap = np.empty(n, np.int64)
    cmap[order_d] = cidx
    rmap = np.empty(n, np.int64)
    for c_ in range(N_CORES):
        mine = order_d[cidx == c_]
        bpos = np.arange(len(mine))
        bcol = bpos % NBLK
        brow = bpos // NBLK
        bidx = np.where(brow % 2 == 0, bcol, NBLK - 1 - bcol)
        rmap[mine] = bidx * DBLK + brow
    core = cmap[rr]
    rloc = rmap[rr]
    invmap = np.full((N_CORES, NBLK * DBLK), -1, np.int64)
    invmap[cmap, rmap] = np.arange(n)

    NS = NKD + 2
    # per (core, sup, stream) edge lists
    order = np.lexsort((cc, rloc, core))
    rr_s, cc_s, rl_s, co_s = rr[order], cc[order], rloc[order], core[order]
    sup_s = rl_s // SUP

    rng = np.random.RandomState(12345)
    kblk = cc_s // KBLK
    # sup 0 stays DMA-only so the pool path never stalls on the resident load
    elig = (cc_s < RES_NODES) & (sup_s >= 1)
    topool = elig & (rng.rand(len(cc_s)) < pool_frac)
    stream = np.where(topool, NKD + cc_s // RES_ELEMS,
                      np.minimum(kblk, NKD - 1))
    # (col >= 98304 goes to k=3 stream; idx offset handled per stream)

    # counts [core, sup, stream]
    idx3 = (co_s * NSUP + sup_s) * NS + stream
    cnt = np.bincount(idx3, minlength=N_CORES * NSUP * NS).reshape(
        N_CORES, NSUP, NS)
    grid = cnt.max(axis=0)          # [NSUP, NS] shared slot allocation
    base = np.zeros((NSUP, NS), np.int64)
    base[1:, :] = np.cumsum(grid, axis=0)[:-1, :]
    L = grid.sum(axis=0)            # stream lengths
    # pad stream lengths: dma to %128, pool to %PB
    Lp = np.empty(NS, np.int64)
    for s in range(NS):
        m = CH if s < NKD else PB
        Lp[s] = max(((L[s] + m - 1) // m) * m, m)

    nchunks = [int(Lp[s] // CH) for s in range(NS)]

    # per-core slot assignment (position within (core, sup, stream) group,
    # preserving the rloc-sorted order within each group)
    gstart = np.zeros(N_CORES * NSUP * NS, np.int64)
    gcnt = np.bincount(idx3, minlength=N_CORES * NSUP * NS)
    gstart[1:] = np.cumsum(gcnt)[:-1]
    order2 = np.argsort(idx3, kind="stable")
    within = np.empty(len(rr_s), np.int64)
    within[order2] = np.arange(len(rr_s)) - gstart[idx3[order2]]
    slot = base[sup_s, stream] + within   # slot within stream

    # events: (stream, chunk, block) union over cores
    # per-core b-ranges per (stream, chunk): compute via edge slots
    bset = set()
    b_s = rl_s // DBLK
    chunk_of = slot // CH
    for key in zip(stream, chunk_of, b_s):
        bset.add(key)
    events = sorted(bset, key=lambda t: (t[2], t[0], t[1]))  # by (b, s, ci)
    NEV = len(events)
    NEVp = ((NEV + SB - 1) // SB) * SB

    # first/last event index per block
    ev_of_b = {}
    for j, (s, ci, b) in enumerate(events):
        ev_of_b.setdefault(b, []).append(j)

    # per-core device arrays
    per_core = []
    ev_index = {(s, ci, b): j for j, (s, ci, b) in enumerate(events)}
    for ci_ in range(N_CORES):
        sel = co_s == ci_
        st_c, sl_c, cc_c, rl_c, b_c = (stream[sel], slot[sel], cc_s[sel],
                                       rl_s[sel], b_s[sel])
        idx_arrs = []
        for s in range(NS):
            arr = np.zeros(int(Lp[s]), np.int16)
            m = st_c == s
            off = (RES_ELEMS * (s - NKD) if s >= NKD
                   else KBLK * min(s, NKD - 1))
            v = cc_c[m] - off
            if s == NKD - 1:  # k=3 stream also holds col >= 98304
                v = np.minimum(v, KBLK - 1)  # safety; cols < 100000-98304+32768 ok
            arr[sl_c[m]] = v.astype(np.int16)
            wrapped = arr.reshape(-1, 16).T.copy()  # [16, Lp/16]
            if s < NKD:
                # ship [16, n] as f32; PE broadcasts to 128 partitions on-chip
                idx_arrs.append(np.ascontiguousarray(
                    wrapped.astype(np.float32)))
            else:
                idx_arrs.append(np.tile(wrapped, (4, 1)).copy())

        rowloc = np.full((NEVp, CH), -1.0, dtype=np.float32)
        jj = np.array([ev_index[(s_, sl_ // CH, b_)]
                       for s_, sl_, b_ in zip(st_c, sl_c, b_c)])
        rowloc[jj, sl_c % CH] = (rl_c - b_c * DBLK).astype(np.float32)
        inv_c = invmap[ci_]
        disfull = np.where(inv_c >= 0, dis[np.maximum(inv_c, 0)], 0.0)
        disdst = disfull.reshape(NBLK, DBLK).T.astype(np.float32).copy()

        per_core.append({
            "rowloc": np.ascontiguousarray(
                rowloc.T.astype(ml_dtypes.bfloat16)),  # [128, NEVp] bf16
            "disdst": disdst,                          # [128, NBLK] f32
            **{f"idx{s}": idx_arrs[s] for s in range(NS)},
        })

    iotab = np.repeat(np.arange(DBLK, dtype=np.float32), SB)[None, :]
    iotab = np.tile(iotab, (128, 1)).astype(ml_dtypes.bfloat16)  # d-major
    shared = {
        "xs": np.ascontiguousarray(xs),
        "resv": resv,
        "wb": weight.astype(ml_dtypes.bfloat16),
        "iotab": iotab,
        "biasrep": np.tile(bias.astype(np.float32)[None, :], (DBLK, 1)),
        "eye64": np.tile(np.eye(64, dtype=np.float32), (2, 1)),
        "sel16": np.ascontiguousarray(
            np.tile(np.eye(16, dtype=np.float32), (1, 8))),
    }
    struct = {
        "events": events, "NEV": NEV, "NEVp": NEVp, "Lp": Lp,
        "nchunks": nchunks, "NS": NS, "cmap": cmap, "rmap": rmap,
    }
    return struct, shared, per_core


def _build(struct):
    events, NEVp, Lp, nchunks, NS = (struct["events"], struct["NEVp"],
                                     struct["Lp"], struct["nchunks"],
                                     struct["NS"])
    nc = bacc.Bacc(None, target_bir_lowering=False,
                   dynamic_dma_scratch_size=65536)
    dt = mybir.dt

    xs_d = nc.dram_tensor("xs", [100000, F], dt.bfloat16, kind="ExternalInput")
    resv_d = nc.dram_tensor("resv", [128, RES_ELEMS], dt.float32,
                            kind="ExternalInput")
    w_d = nc.dram_tensor("wb", [F, F], dt.bfloat16, kind="ExternalInput")
    iota_d = nc.dram_tensor("iotab", [128, SB * DBLK], dt.bfloat16,
                            kind="ExternalInput")
    bias_d = nc.dram_tensor("biasrep", [DBLK, F], dt.float32,
                            kind="ExternalInput")
    rowloc_d = nc.dram_tensor("rowloc", [128, NEVp], dt.bfloat16,
                              kind="ExternalInput")
    disdst_d = nc.dram_tensor("disdst", [DBLK, NBLK], dt.float32,
                              kind="ExternalInput")
    eye_d = nc.dram_tensor("eye64", [128, 64], dt.float32,
                           kind="ExternalInput")
    sel_d = nc.dram_tensor("sel16", [16, 128], dt.float32,
                           kind="ExternalInput")
    idx_d = [nc.dram_tensor(f"idx{s}", [16, int(Lp[s]) // 16], dt.float32,
                            kind="ExternalInput") if s < NKD else
             nc.dram_tensor(f"idx{s}", [64, int(Lp[s]) // 16], dt.int16,
                            kind="ExternalInput")
             for s in range(NS)]
    y_d = nc.dram_tensor("y", [NBLK * DBLK, F], dt.float32,
                         kind="ExternalOutput")

    CPB_D = GB // CH   # dma chunks per slab batch
    CPB_P = PB // CH   # pool chunks per batch
    have_pool = any(s >= NKD for s, _, _ in events)

    with tile.TileContext(nc) as tc:
        with (
            tc.tile_pool(name="const", bufs=1) as constp,
            tc.tile_pool(name="slab", bufs=3) as slabp,
            tc.tile_pool(name="idxp", bufs=3) as idxp,
            tc.tile_pool(name="idxfp", bufs=3) as idxfp,
            tc.tile_pool(name="ibcp", bufs=2, space="PSUM") as ibcp,
            tc.tile_pool(name="pidxp", bufs=2) as pidxp,
            tc.tile_pool(name="slabt", bufs=2) as slabtp,
            tc.tile_pool(name="slab8", bufs=3) as slab8p,
            tc.tile_pool(name="sp", bufs=3) as sp_,
            tc.tile_pool(name="pre", bufs=2) as prep,
            tc.tile_pool(name="ob", bufs=2) as obp,
            tc.tile_pool(name="ptr", bufs=2, space="PSUM") as ptrp,
            tc.tile_pool(name="pagg", bufs=4, space="PSUM") as paggp,
            tc.tile_pool(name="pout", bufs=2, space="PSUM") as poutp,
        ):
            if have_pool:
                nc.gpsimd.load_library(library_config.ap_gather)
            x_src = [xs_d[min(s, NKD - 1) * KBLK:
                          min((min(s, NKD - 1) + 1) * KBLK, 100000), :]
                     for s in range(NKD)]

            sel_sb = constp.tile([16, 128], dt.float32, tag="sel16")
            nc.sync.dma_start(sel_sb[:], sel_d[:])

            # stream state
            nbatch_done = [0] * NS
            slab_tiles = {}   # (s, batch) -> (tile, kind)

            def ensure_batch(s, bi, prefetch=True):
                if nbatch_done[s] > bi:
                    return
                assert nbatch_done[s] == bi, (s, bi, nbatch_done[s])
                nbatch_done[s] += 1
                nbatches_s = (int(Lp[s]) + (GB if s < NKD else PB) - 1) \
                    // (GB if s < NKD else PB)
                if s < NKD:
                    n_idx = min(GB, int(Lp[s]) - bi * GB)
                    n_ch = (n_idx + CH - 1) // CH
                    slab = slabp.tile([128, CPB_D, F], dt.bfloat16,
                                      tag=f"slab{s}")
                    itf = idxfp.tile([16, GB // 16], dt.float32,
                                     tag=f"idxf{s}")
                    nc.sync.dma_start(
                        itf[:, :n_idx // 16],
                        idx_d[s][:, bi * (GB // 16):
                                 bi * (GB // 16) + n_idx // 16])
                    ibc = ibcp.tile([128, GB // 16], dt.float32, tag="ibc")
                    nc.tensor.matmul(ibc[:, :n_idx // 16], sel_sb[:],
                                     itf[:, :n_idx // 16],
                                     start=True, stop=True)
                    it = idxp.tile([128, GB // 16], dt.int16, tag=f"idx{s}")
                    nc.vector.tensor_copy(it[:, :n_idx // 16],
                                          ibc[:, :n_idx // 16])
                    nc.gpsimd.dma_gather(
                        slab[:, :n_ch, :], x_src[s], it[:, :n_idx // 16],
                        n_idx, n_idx, F, single_packet=False)
                    slab_tiles[(s, bi)] = slab
                    if prefetch and bi + 1 < nbatches_s:
                        ensure_batch(s, bi + 1, prefetch=False)
                else:
                    h = s - NKD
                    pt_full = pidxp.tile([128, PB // 16], dt.int16,
                                         tag=f"pidx{h}")
                    pt = pt_full[h * 64:(h + 1) * 64, :]
                    nc.sync.dma_start(
                        pt, idx_d[s][:, bi * (PB // 16):
                                     (bi + 1) * (PB // 16)])
                    st = slabtp.tile([128, PB], dt.float32, tag=f"slabt{h}")
                    sl = st[h * 64:(h + 1) * 64, :]
                    nc.gpsimd.ap_gather(
                        sl, res_sb[h * 64:(h + 1) * 64, :], pt,
                        64, RES_ELEMS, 1, PB)
                    ptr = ptrp.tile([128, CPB_P, 64], dt.float32,
                                    tag="ptr")
                    for w in range(CPB_P):
                        nc.tensor.transpose(
                            ptr[:, w, :], sl[:, w * CH:(w + 1) * CH],
                            eye_sb[h * 64:(h + 1) * 64, :])
                    s8 = slab8p.tile([128, CPB_P * 64], dt.float32,
                                     tag=f"s8_{h}")
                    nc.scalar.activation(
                        s8[:], ptr[:].rearrange("p a b -> p (a b)"),
                        mybir.ActivationFunctionType.Copy)
                    slab_tiles[(s, bi)] = s8

            def slab_slice(s, ci):
                cpb = CPB_D if s < NKD else CPB_P
                bi = ci // cpb
                t = slab_tiles[(s, bi)]
                if s < NKD:
                    return t[:, ci % cpb, :]
                w = ci % cpb
                return t[:, w * 64:(w + 1) * 64].bitcast(dt.bfloat16)

            # prefetch the first gather batch of every dma stream so the
            # DMA device starts on edge data, not consts
            for s_ in range(NKD):
                if int(Lp[s_]) > 0 and nchunks[s_] > 0:
                    ensure_batch(s_, 0)

            w_sb = constp.tile([F, F], dt.bfloat16, tag="w")
            nc.sync.dma_start(w_sb[:], w_d[:])
            iota_sb = constp.tile([128, SB * DBLK], dt.bfloat16, tag="iota")
            nc.sync.dma_start(iota_sb[:], iota_d[:])
            bias_sb = constp.tile([DBLK, F], dt.float32, tag="bias")
            nc.sync.dma_start(bias_sb[:], bias_d[:])
            rowloc_sb = constp.tile([128, NEVp], dt.bfloat16, tag="rowloc")
            nc.sync.dma_start(rowloc_sb[:], rowloc_d[:])
            disdst_sb = constp.tile([DBLK, NBLK], dt.float32, tag="disdst")
            nc.sync.dma_start(disdst_sb[:], disdst_d[:])
            if have_pool:
                res_sb = constp.tile([128, RES_ELEMS], dt.float32, tag="res")
                nc.sync.dma_start(res_sb[:], resv_d[:])
                eye_sb = constp.tile([128, 64], dt.float32, tag="eye")
                nc.sync.dma_start(eye_sb[:], eye_d[:])


            # block -> event index span (events sorted by (b, s, ci))
            ev_b = [e[2] for e in events]
            s8_tiles = {}

            def ensure_s8(g):
                if g in s8_tiles:
                    return
                t = sp_.tile([128, DBLK * SB], dt.bfloat16, tag="s8t")
                nc.vector.tensor_tensor(
                    t[:].rearrange("p (a b) -> p a b", a=DBLK),
                    iota_sb[:].rearrange("p (a b) -> p a b", a=DBLK),
                    rowloc_sb[:, g * SB:(g + 1) * SB]
                    .unsqueeze(1).broadcast_to([128, DBLK, SB]),
                    mybir.AluOpType.is_equal)
                s8_tiles[g] = t

            j = 0
            NEV = struct["NEV"]
            while j < NEV:
                b = ev_b[j]
                j_end = j
                while j_end < NEV and ev_b[j_end] == b:
                    j_end += 1
                # pre-pass: all gathers/transposes/copies/S-builds for this
                # block BEFORE opening the PE accumulation group (a PE
                # transpose inside an open group deadlocks the PE order)
                for jj in range(j, j_end):
                    s, ci, _ = events[jj]
                    cpb = CPB_D if s < NKD else CPB_P
                    ensure_batch(s, ci // cpb)
                pa = paggp.tile([128, DBLK], dt.float32, tag="pagg")
                for jj in range(j, j_end):
                    s, ci, _ = events[jj]
                    ensure_s8(jj // SB)
                    st8 = s8_tiles[jj // SB]
                    nc.tensor.matmul(
                        pa[:], slab_slice(s, ci),
                        st8[:].rearrange("p (a b) -> p a b", a=DBLK)
                        [:, :, jj % SB],
                        start=(jj == j), stop=(jj == j_end - 1))
                # drain block b
                pre = prep.tile([128, DBLK], dt.bfloat16, tag="pre")
                nc.scalar.activation(pre[:], pa[:],
                                     mybir.ActivationFunctionType.Copy)
                po = poutp.tile([DBLK, F], dt.float32, tag="po")
                nc.tensor.matmul(po[:], pre[:], w_sb[:], start=True,
                                 stop=True)
                ob = obp.tile([DBLK, F], dt.float32, tag="ob")
                nc.vector.scalar_tensor_tensor(
                    ob[:], po[:], disdst_sb[:, b:b + 1], bias_sb[:],
                    op0=mybir.AluOpType.mult, op1=mybir.AluOpType.add)
                nc.sync.dma_start(y_d[b * DBLK:(b + 1) * DBLK, :], ob[:])
                j = j_end

    nc.compile()
    return nc


def kernel(x, edge_index, weight, bias, _pool_frac=0.0, _return_nc=False):
    x = np.ascontiguousarray(np.asarray(x, dtype=np.float32))
    edge_index = np.asarray(edge_index)
    weight = np.ascontiguousarray(np.asarray(weight, dtype=np.float32))
    bias = np.asarray(bias, dtype=np.float32)
    n = x.shape[0]
    assert n == 100000 and n % N_CORES == 0

    struct, shared, per_core = _prep(x, edge_index, weight, bias, _pool_frac)
    nc = _build(struct)

    in_maps = [{**shared, **per_core[ci]} for ci in range(N_CORES)]
    res = run_bass_kernel_spmd(nc, in_maps, core_ids=list(range(N_CORES)))
    ys = np.stack([np.asarray(res.results[ci]["y"]).astype(np.float32)
                   for ci in range(N_CORES)])
    out = ys[struct["cmap"], struct["rmap"]]
    if _return_nc:
        return out, nc, in_maps
    return out


# revision 37
# speedup vs baseline: 1.1026x; 1.0607x over previous
"""GCN layer on 8 trn2 NeuronCores -- flat-stream bf16 gather pipeline.

out[r] = (sum_{e:row[e]=r} dis[row_e]*dis[col_e] * x[col_e]) @ W + bias,
dis = rsqrt(1 + outdeg), self-loops included as ordinary edges.

Sharding: destination nodes partitioned across 8 cores (12500 each),
degree-serpentine balanced; each core independent (x replicated; no
collectives). Host unshards via the (core, rloc) permutation.

Device algorithm per core (norm fully factored out of the edge stream):
  - X' = x * dis[col] in bf16 (host). dis[row] applied at the output stage,
    so the edge stream carries no per-edge scaling at all.
  - Edges land in 4 flat gather streams by col-block of 32768 (int16 idx
    limit), sorted by dest; slots are a per-(512-dest superblock, stream)
    grid equalized across cores so one NEFF serves all 8. Padding ~1.7%.
  - dma_gather in 3584-edge batches (amortizes the 994ns SWDGE overhead;
    64KB/partition SWDGE scratch -- an 8KB ring silently corrupts on HW)
    pulls 256B bf16 rows into [128, 28, 128] slabs, 2 buffers per stream.
  - Per 128-slot chunk and touched dest block: S = (iota == rowloc) 0/1
    indicator bf16, built 8 chunks per DVE tensor_tensor (d-major layout
    keeps every operand's last dim stride-1 so DVE runs in 2x mode; the
    rowloc broadcast rides the middle dim). Chunks may straddle block
    boundaries (the straddling chunk matmuls twice) -- no per-group chunk
    rounding, so DMA pays only real slots.
  - PE accumulates aggT[f,d] += slab.T @ S in PSUM per 128-dest block.
  - Drain per block: aggT->bf16 (Act), po = aggT.T @ W (PE),
    ob = po*dis_d + bias (one DVE scalar_tensor_tensor), DMA to y.

Cost-model floor: the gather descriptors (256B -> 22.76ns each, /16 DMA
engines) put the DMA device at ~597us busy; DVE ~50%, PE ~47%, Pool ~39%.
The GPSIMD ap_gather offload path was explored and abandoned: its cost is
max(num_idxs, num_elems) per call, which the SBUF-resident table dominates.
"""

import numpy as np
import ml_dtypes

import concourse.bass as bass
import concourse.mybir as mybir
import concourse.tile as tile
from concourse import bacc
from concourse import library_config
from concourse.bass_utils import run_bass_kernel_spmd

F = 128
CH = 128          # edges per chunk (slab partition dim)
N_CORES = 8
NPC = 12500       # dest nodes per core
DBLK = 128        # dest block width (psum tile)
NBLK = (NPC + DBLK - 1) // DBLK   # 98
SUP = 512         # superblock of dests (grid granularity)
NSUP = (NPC + SUP - 1) // SUP     # 25
KBLK = 32768      # dma-path col blocking (int16 idx limit)
NKD = 4           # dma streams
RES_ELEMS = 24576  # ap_gather num_elems per half (f32 pair elements)
RES_NODES = 2 * RES_ELEMS  # 65536 cols covered by the resident
SB = 8            # chunks per S8 tensor_tensor batch
GB = 3584         # edges per dma_gather batch
PB = 1024         # edges per ap_gather batch (8 transpose windows)


def _prep(x, edge_index, weight, bias, pool_frac):
    """Host-side routing/index prep. Returns (shared structure, per-core arrays)."""
    n = x.shape[0]
    r = np.asarray(edge_index[0], dtype=np.int64)
    c = np.asarray(edge_index[1], dtype=np.int64)
    deg = (np.bincount(r, minlength=n) + 1).astype(np.float64)
    dis = (1.0 / np.sqrt(deg)).astype(np.float32)

    loops = np.arange(n, dtype=np.int64)
    rr = np.concatenate([r, loops])
    cc = np.concatenate([c, loops])

    xs = (x * dis[:, None]).astype(ml_dtypes.bfloat16)   # X' [n, 128] bf16
    # pair-packed resident: [128, RES_ELEMS] f32; partition p (=64*h+q) holds
    # f32 word q of X'[u] for nodes u in half h's range split across q... see
    # below: half h serves cols [h*32768, (h+1)*32768), channels h*64..h*64+63,
    # element u = node h*32768+u, partition q = feature pair q.
    resv = np.zeros((128, RES_ELEMS), dtype=np.float32)
    for h in range(2):
        blk = xs[h * RES_ELEMS:(h + 1) * RES_ELEMS].view(np.float32)
        resv[h * 64:(h + 1) * 64, :] = blk.T  # [64, RES_ELEMS]

    # degree-balanced dest -> (core, rloc) assignment: serpentine by degree
    # equalizes per-(core, sup, stream) counts so the cross-core max grid
    # carries ~1% padding instead of ~4.6%
    order_d = np.argsort(-deg, kind="stable")
    pos = np.arange(n)
    colc = pos % N_CORES
    cidx = np.where((pos // N_CORES) % 2 == 0, colc, N_CORES - 1 - colc)
    cmap = np.empty(n, np.int64)
    cmap[order_d] = cidx
    rmap = np.empty(n, np.int64)
    for c_ in range(N_CORES):
        mine = order_d[cidx == c_]
        bpos = np.arange(len(mine))
        bcol = bpos % NBLK
        brow = bpos // NBLK
        bidx = np.where(brow % 2 == 0, bcol, NBLK - 1 - bcol)
        rmap[mine] = bidx * DBLK + brow
    core = cmap[rr]
    rloc = rmap[rr]
    invmap = np.full((N_CORES, NBLK * DBLK), -1, np.int64)
    invmap[cmap, rmap] = np.arange(n)

    NS = NKD + 2
    # per (core, sup, stream) edge lists
    order = np.lexsort((cc, rloc, core))
    rr_s, cc_s, rl_s, co_s = rr[order], cc[order], rloc[order], core[order]
    sup_s = rl_s // SUP

    rng = np.random.RandomState(12345)
    kblk = cc_s // KBLK
    # sup 0 stays DMA-only so the pool path never stalls on the resident load
    elig = (cc_s < RES_NODES) & (sup_s >= 1)
    topool = elig & (rng.rand(len(cc_s)) < pool_frac)
    stream = np.where(topool, NKD + cc_s // RES_ELEMS,
                      np.minimum(kblk, NKD - 1))
    # (col >= 98304 goes to k=3 stream; idx offset handled per stream)

    # counts [core, sup, stream]
    idx3 = (co_s * NSUP + sup_s) * NS + stream
    cnt = np.bincount(idx3, minlength=N_CORES * NSUP * NS).reshape(
        N_CORES, NSUP, NS)
    grid = cnt.max(axis=0)          # [NSUP, NS] shared slot allocation
    base = np.zeros((NSUP, NS), np.int64)
    base[1:, :] = np.cumsum(grid, axis=0)[:-1, :]
    L = grid.sum(axis=0)            # stream lengths
    # pad stream lengths: dma to %128, pool to %PB
    Lp = np.empty(NS, np.int64)
    for s in range(NS):
        m = CH if s < NKD else PB
        Lp[s] = max(((L[s] + m - 1) // m) * m, m)

    nchunks = [int(Lp[s] // CH) for s in range(NS)]

    # per-core slot assignment (position within (core, sup, stream) group,
    # preserving the rloc-sorted order within each group)
    gstart = np.zeros(N_CORES * NSUP * NS, np.int64)
    gcnt = np.bincount(idx3, minlength=N_CORES * NSUP * NS)
    gstart[1:] = np.cumsum(gcnt)[:-1]
    order2 = np.argsort(idx3, kind="stable")
    within = np.empty(len(rr_s), np.int64)
    within[order2] = np.arange(len(rr_s)) - gstart[idx3[order2]]
    slot = base[sup_s, stream] + within   # slot within stream

    # events: (stream, chunk, block) union over cores
    # per-core b-ranges per (stream, chunk): compute via edge slots
    bset = set()
    b_s = rl_s // DBLK
    chunk_of = slot // CH
    for key in zip(stream, chunk_of, b_s):
        bset.add(key)
    events = sorted(bset, key=lambda t: (t[2], t[0], t[1]))  # by (b, s, ci)
    NEV = len(events)
    NEVp = ((NEV + SB - 1) // SB) * SB

    # first/last event index per block
    ev_of_b = {}
    for j, (s, ci, b) in enumerate(events):
        ev_of_b.setdefault(b, []).append(j)

    # per-core device arrays
    per_core = []
    ev_index = {(s, ci, b): j for j, (s, ci, b) in enumerate(events)}
    for ci_ in range(N_CORES):
        sel = co_s == ci_
        st_c, sl_c, cc_c, rl_c, b_c = (stream[sel], slot[sel], cc_s[sel],
                                       rl_s[sel], b_s[sel])
        idx_arrs = []
        for s in range(NS):
            arr = np.zeros(int(Lp[s]), np.int16)
            m = st_c == s
            off = (RES_ELEMS * (s - NKD) if s >= NKD
                   else KBLK * min(s, NKD - 1))
            v = cc_c[m] - off
            if s == NKD - 1:  # k=3 stream also holds col >= 98304
                v = np.minimum(v, KBLK - 1)  # safety; cols < 100000-98304+32768 ok
            arr[sl_c[m]] = v.astype(np.int16)
            # wrap 16 partitions, replicate across the 8 gpsimd cores
            wrapped = arr.reshape(-1, 16).T.copy()  # [16, Lp/16]
            rep = 8 if s < NKD else 4
            idx_arrs.append(np.tile(wrapped, (rep, 1)).copy())

        rowloc = np.full((NEVp, CH), -1.0, dtype=np.float32)
        jj = np.array([ev_index[(s_, sl_ // CH, b_)]
                       for s_, sl_, b_ in zip(st_c, sl_c, b_c)])
        rowloc[jj, sl_c % CH] = (rl_c - b_c * DBLK).astype(np.float32)
        inv_c = invmap[ci_]
        disfull = np.where(inv_c >= 0, dis[np.maximum(inv_c, 0)], 0.0)
        disdst = disfull.reshape(NBLK, DBLK).T.astype(np.float32).copy()

        per_core.append({
            "rowloc": np.ascontiguousarray(
                rowloc.T.astype(ml_dtypes.bfloat16)),  # [128, NEVp] bf16
            "disdst": disdst,                          # [128, NBLK] f32
            **{f"idx{s}": idx_arrs[s] for s in range(NS)},
        })

    iotab = np.repeat(np.arange(DBLK, dtype=np.float32), SB)[None, :]
    iotab = np.tile(iotab, (128, 1)).astype(ml_dtypes.bfloat16)  # d-major
    shared = {
        "xs": np.ascontiguousarray(xs),
        "resv": resv,
        "wb": weight.astype(ml_dtypes.bfloat16),
        "iotab": iotab,
        "biasrep": np.tile(bias.astype(np.float32)[None, :], (DBLK, 1)),
        "eye64": np.tile(np.eye(64, dtype=np.float32), (2, 1)),
    }
    struct = {
        "events": events, "NEV": NEV, "NEVp": NEVp, "Lp": Lp,
        "nchunks": nchunks, "NS": NS, "cmap": cmap, "rmap": rmap,
    }
    return struct, shared, per_core


def _build(struct):
    events, NEVp, Lp, nchunks, NS = (struct["events"], struct["NEVp"],
                                     struct["Lp"], struct["nchunks"],
                                     struct["NS"])
    nc = bacc.Bacc(None, target_bir_lowering=False,
                   dynamic_dma_scratch_size=65536)
    dt = mybir.dt

    xs_d = nc.dram_tensor("xs", [100000, F], dt.bfloat16, kind="ExternalInput")
    resv_d = nc.dram_tensor("resv", [128, RES_ELEMS], dt.float32,
                            kind="ExternalInput")
    w_d = nc.dram_tensor("wb", [F, F], dt.bfloat16, kind="ExternalInput")
    iota_d = nc.dram_tensor("iotab", [128, SB * DBLK], dt.bfloat16,
                            kind="ExternalInput")
    bias_d = nc.dram_tensor("biasrep", [DBLK, F], dt.float32,
                            kind="ExternalInput")
    rowloc_d = nc.dram_tensor("rowloc", [128, NEVp], dt.bfloat16,
                              kind="ExternalInput")
    disdst_d = nc.dram_tensor("disdst", [DBLK, NBLK], dt.float32,
                              kind="ExternalInput")
    eye_d = nc.dram_tensor("eye64", [128, 64], dt.float32,
                           kind="ExternalInput")
    idx_d = [nc.dram_tensor(f"idx{s}", [128 if s < NKD else 64,
                                        int(Lp[s]) // 16], dt.int16,
                            kind="ExternalInput")
             for s in range(NS)]
    y_d = nc.dram_tensor("y", [NBLK * DBLK, F], dt.float32,
                         kind="ExternalOutput")

    CPB_D = GB // CH   # dma chunks per slab batch
    CPB_P = PB // CH   # pool chunks per batch
    have_pool = any(s >= NKD for s, _, _ in events)

    with tile.TileContext(nc) as tc:
        with (
            tc.tile_pool(name="const", bufs=1) as constp,
            tc.tile_pool(name="slab", bufs=2) as slabp,
            tc.tile_pool(name="idxp", bufs=2) as idxp,
            tc.tile_pool(name="pidxp", bufs=2) as pidxp,
            tc.tile_pool(name="slabt", bufs=2) as slabtp,
            tc.tile_pool(name="slab8", bufs=3) as slab8p,
            tc.tile_pool(name="sp", bufs=3) as sp_,
            tc.tile_pool(name="pre", bufs=2) as prep,
            tc.tile_pool(name="ob", bufs=2) as obp,
            tc.tile_pool(name="ptr", bufs=2, space="PSUM") as ptrp,
            tc.tile_pool(name="pagg", bufs=4, space="PSUM") as paggp,
            tc.tile_pool(name="pout", bufs=2, space="PSUM") as poutp,
        ):
            if have_pool:
                nc.gpsimd.load_library(library_config.ap_gather)
            x_src = [xs_d[min(s, NKD - 1) * KBLK:
                          min((min(s, NKD - 1) + 1) * KBLK, 100000), :]
                     for s in range(NKD)]

            # stream state
            nbatch_done = [0] * NS
            slab_tiles = {}   # (s, batch) -> (tile, kind)

            def ensure_batch(s, bi):
                if nbatch_done[s] > bi:
                    return
                assert nbatch_done[s] == bi, (s, bi, nbatch_done[s])
                nbatch_done[s] += 1
                if s < NKD:
                    n_idx = min(GB, int(Lp[s]) - bi * GB)
                    n_ch = (n_idx + CH - 1) // CH
                    slab = slabp.tile([128, CPB_D, F], dt.bfloat16,
                                      tag=f"slab{s}")
                    it = idxp.tile([128, GB // 16], dt.int16, tag=f"idx{s}")
                    nc.sync.dma_start(
                        it[:, :n_idx // 16],
                        idx_d[s][:, bi * (GB // 16):
                                 bi * (GB // 16) + n_idx // 16])
                    nc.gpsimd.dma_gather(
                        slab[:, :n_ch, :], x_src[s], it[:, :n_idx // 16],
                        n_idx, n_idx, F, single_packet=False)
                    slab_tiles[(s, bi)] = slab
                else:
                    h = s - NKD
                    pt_full = pidxp.tile([128, PB // 16], dt.int16,
                                         tag=f"pidx{h}")
                    pt = pt_full[h * 64:(h + 1) * 64, :]
                    nc.sync.dma_start(
                        pt, idx_d[s][:, bi * (PB // 16):
                                     (bi + 1) * (PB // 16)])
                    st = slabtp.tile([128, PB], dt.float32, tag=f"slabt{h}")
                    sl = st[h * 64:(h + 1) * 64, :]
                    nc.gpsimd.ap_gather(
                        sl, res_sb[h * 64:(h + 1) * 64, :], pt,
                        64, RES_ELEMS, 1, PB)
                    ptr = ptrp.tile([128, CPB_P, 64], dt.float32,
                                    tag="ptr")
                    for w in range(CPB_P):
                        nc.tensor.transpose(
                            ptr[:, w, :], sl[:, w * CH:(w + 1) * CH],
                            eye_sb[h * 64:(h + 1) * 64, :])
                    s8 = slab8p.tile([128, CPB_P * 64], dt.float32,
                                     tag=f"s8_{h}")
                    nc.scalar.activation(
                        s8[:], ptr[:].rearrange("p a b -> p (a b)"),
                        mybir.ActivationFunctionType.Copy)
                    slab_tiles[(s, bi)] = s8

            def slab_slice(s, ci):
                cpb = CPB_D if s < NKD else CPB_P
                bi = ci // cpb
                t = slab_tiles[(s, bi)]
                if s < NKD:
                    return t[:, ci % cpb, :]
                w = ci % cpb
                return t[:, w * 64:(w + 1) * 64].bitcast(dt.bfloat16)

            # prefetch the first gather batch of every dma stream so the
            # DMA device starts on edge data, not consts
            for s_ in range(NKD):
                if int(Lp[s_]) > 0 and nchunks[s_] > 0:
                    ensure_batch(s_, 0)

            w_sb = constp.tile([F, F], dt.bfloat16, tag="w")
            nc.sync.dma_start(w_sb[:], w_d[:])
            iota_sb = constp.tile([128, SB * DBLK], dt.bfloat16, tag="iota")
            nc.sync.dma_start(iota_sb[:], iota_d[:])
            bias_sb = constp.tile([DBLK, F], dt.float32, tag="bias")
            nc.sync.dma_start(bias_sb[:], bias_d[:])
            rowloc_sb = constp.tile([128, NEVp], dt.bfloat16, tag="rowloc")
            nc.sync.dma_start(rowloc_sb[:], rowloc_d[:])
            disdst_sb = constp.tile([DBLK, NBLK], dt.float32, tag="disdst")
            nc.sync.dma_start(disdst_sb[:], disdst_d[:])
            if have_pool:
                res_sb = constp.tile([128, RES_ELEMS], dt.float32, tag="res")
                nc.sync.dma_start(res_sb[:], resv_d[:])
                eye_sb = constp.tile([128, 64], dt.float32, tag="eye")
                nc.sync.dma_start(eye_sb[:], eye_d[:])


            # block -> event index span (events sorted by (b, s, ci))
            ev_b = [e[2] for e in events]
            s8_tiles = {}

            def ensure_s8(g):
                if g in s8_tiles:
                    return
                t = sp_.tile([128, DBLK * SB], dt.bfloat16, tag="s8t")
                nc.vector.tensor_tensor(
                    t[:].rearrange("p (a b) -> p a b", a=DBLK),
                    iota_sb[:].rearrange("p (a b) -> p a b", a=DBLK),
                    rowloc_sb[:, g * SB:(g + 1) * SB]
                    .unsqueeze(1).broadcast_to([128, DBLK, SB]),
                    mybir.AluOpType.is_equal)
                s8_tiles[g] = t

            j = 0
            NEV = struct["NEV"]
            while j < NEV:
                b = ev_b[j]
                j_end = j
                while j_end < NEV and ev_b[j_end] == b:
                    j_end += 1
                # pre-pass: all gathers/transposes/copies/S-builds for this
                # block BEFORE opening the PE accumulation group (a PE
                # transpose inside an open group deadlocks the PE order)
                for jj in range(j, j_end):
                    s, ci, _ = events[jj]
                    cpb = CPB_D if s < NKD else CPB_P
                    ensure_batch(s, ci // cpb)
                pa = paggp.tile([128, DBLK], dt.float32, tag="pagg")
                for jj in range(j, j_end):
                    s, ci, _ = events[jj]
                    ensure_s8(jj // SB)
                    st8 = s8_tiles[jj // SB]
                    nc.tensor.matmul(
                        pa[:], slab_slice(s, ci),
                        st8[:].rearrange("p (a b) -> p a b", a=DBLK)
                        [:, :, jj % SB],
                        start=(jj == j), stop=(jj == j_end - 1))
                # drain block b
                pre = prep.tile([128, DBLK], dt.bfloat16, tag="pre")
                nc.scalar.activation(pre[:], pa[:],
                                     mybir.ActivationFunctionType.Copy)
                po = poutp.tile([DBLK, F], dt.float32, tag="po")
                nc.tensor.matmul(po[:], pre[:], w_sb[:], start=True,
                                 stop=True)
                ob = obp.tile([DBLK, F], dt.float32, tag="ob")
                nc.vector.scalar_tensor_tensor(
                    ob[:], po[:], disdst_sb[:, b:b + 1], bias_sb[:],
                    op0=mybir.AluOpType.mult, op1=mybir.AluOpType.add)
                nc.sync.dma_start(y_d[b * DBLK:(b + 1) * DBLK, :], ob[:])
                j = j_end

    nc.compile()
    return nc


def kernel(x, edge_index, weight, bias, _pool_frac=0.0, _return_nc=False):
    x = np.ascontiguousarray(np.asarray(x, dtype=np.float32))
    edge_index = np.asarray(edge_index)
    weight = np.ascontiguousarray(np.asarray(weight, dtype=np.float32))
    bias = np.asarray(bias, dtype=np.float32)
    n = x.shape[0]
    assert n == 100000 and n % N_CORES == 0

    struct, shared, per_core = _prep(x, edge_index, weight, bias, _pool_frac)
    nc = _build(struct)

    in_maps = [{**shared, **per_core[ci]} for ci in range(N_CORES)]
    res = run_bass_kernel_spmd(nc, in_maps, core_ids=list(range(N_CORES)))
    ys = np.stack([np.asarray(res.results[ci]["y"]).astype(np.float32)
                   for ci in range(N_CORES)])
    out = ys[struct["cmap"], struct["rmap"]]
    if _return_nc:
        return out, nc, in_maps
    return out
